# revision 29
# baseline (speedup 1.0000x reference)
"""Self-contained distributed Bass kernel for nn_Atom_Gloal_37958920962359.

Two-layer GCN (PyG GCNConv semantics) + batchnorm + global max pool over
200k nodes / 800k edges / 8192 graphs, plus a cell-line MLP branch, running
SPMD on 8 TRN2 NeuronCores.

v2 design (vs baseline):
- Node columns per core laid out in pool-bucket order: graphs sorted by
  ceil(n/8), each padded to k*8 columns with duplicates of a low-degree
  member node, dummy graphs up to shared per-bucket caps. Max-pool then
  becomes a handful of uniform strided max-reduces over the SBUF-resident
  transposed layer-2 output; no gather, no DRAM round trip.
- GCN aggregation: out = [Dinv (A+I) Dinv x] @ W + b. Slots (edges + self
  loops + duplicate-column copies) are grouped by 64-wide dst windows,
  padded to 128-slot chunks (chunk counts shared across cores for SPMD).
  Gathered rows arrive in bf16 via one batched indirect DMA per 2 dst
  tiles; one-hot scale matrices are built in 2 DVE ops per tile; all
  matmuls run in bf16 (4x PE rate + fast weight load).
- BN stats: dummy/tail columns contribute relu(bias) and are corrected
  exactly; duplicate columns bias the batch stats by ~0.1% (accepted).
- L1 output table is written bf16 and AllGathered bf16 (half the bytes).
"""
import sys
sys.path.insert(0, "/opt/trn_rl_repo")

import numpy as np
from contextlib import ExitStack

import concourse.bass as bass
import concourse.bacc as bacc
import concourse.mybir as mybir
import concourse.tile as tile
from concourse.masks import make_identity
from concourse.bass_utils import run_bass_kernel_spmd


N_NODES = 200000
N_EDGES = 800000
N_GRAPHS = 8192
DIM_DRUG = 128
HID = 128
OUT = 256
DIM_CELL = 954
DIM_CELL_PAD = 1024
EPS = 1e-5
N_CORES = 8
TILE_DST = 256       # dst columns per psum tile
WIN = 64             # one-hot window width
CHUNK = 128          # slots per matmul chunk (K dim)
GPC = N_GRAPHS // N_CORES  # graphs per core
POOL_SUB = 2         # graph column padding granularity
GATHER_TILES = 2     # dst tiles per indirect gather op


# ---------------------------------------------------------------------------
# host-side planning
# ---------------------------------------------------------------------------

def build_plan(drug_adj, ibatch):
    """All integer/index preprocessing. Float math on host is limited to
    graph normalization constants (dinv products, s-vector)."""
    ibatch = np.asarray(ibatch)
    src_all = np.asarray(drug_adj[0]).astype(np.int64)
    dst_all = np.asarray(drug_adj[1]).astype(np.int64)

    # graph-aligned node ranges per core
    node_start = np.searchsorted(ibatch, np.arange(N_CORES + 1) * GPC).astype(np.int64)
    gcnt = np.bincount(ibatch, minlength=N_GRAPHS)

    # degrees including self loop
    deg = np.bincount(dst_all, minlength=N_NODES).astype(np.int64) + 1
    dinv = 1.0 / np.sqrt(deg.astype(np.float64))

    owner_of_node = np.searchsorted(node_start, np.arange(N_NODES), side="right") - 1

    # ---- pool-bucket column layout (shared caps across cores) ----
    kg_all = np.ceil(gcnt / POOL_SUB).astype(np.int64)  # 0 for empty graphs
    KMAX = int(kg_all.max())
    caps = np.zeros(KMAX + 1, dtype=np.int64)
    for c in range(N_CORES):
        kgc = kg_all[c * GPC:(c + 1) * GPC]
        for k in range(1, KMAX + 1):
            caps[k] = max(caps[k], int((kgc == k).sum()))
    G_PAD = int(caps.sum())           # pooled columns per core
    used_cols = int((caps * np.arange(KMAX + 1) * POOL_SUB).sum())
    SH = int(np.ceil(used_cols / TILE_DST) * TILE_DST)
    NT = SH // TILE_DST
    NW = SH // WIN

    # bucket column offsets (same for all cores)
    bucket_col = np.zeros(KMAX + 2, dtype=np.int64)
    for k in range(1, KMAX + 1):
        bucket_col[k + 1] = bucket_col[k] + caps[k] * k * POOL_SUB
    bucket_g = np.zeros(KMAX + 2, dtype=np.int64)
    for k in range(1, KMAX + 1):
        bucket_g[k + 1] = bucket_g[k] + caps[k]

    # ---- per-core node column assignment ----
    cores = []
    total_dup = 0
    for c in range(N_CORES):
        g0, lo, hi = c * GPC, node_start[c], node_start[c + 1]
        kgc = kg_all[g0:g0 + GPC]
        local_ib = ibatch[lo:hi]
        gs = np.searchsorted(local_ib, np.arange(g0, g0 + GPC + 1))

        col_of_node = np.full(hi - lo, -1, dtype=np.int64)   # local node -> col
        node_of_col = np.full(SH, -1, dtype=np.int64)        # col -> local node (-1 dummy)
        gorder = np.full(G_PAD, -1, dtype=np.int64)          # pooled slot -> local graph
        next_in_bucket = np.zeros(KMAX + 1, dtype=np.int64)
        for g in range(GPC):
            k = kgc[g]
            if k == 0:
                continue
            slot = bucket_g[k] + next_in_bucket[k]
            base = bucket_col[k] + next_in_bucket[k] * k * POOL_SUB
            next_in_bucket[k] += 1
            gorder[slot] = g
            a, b = gs[g], gs[g + 1]
            n = b - a
            col_of_node[a:b] = base + np.arange(n)
            node_of_col[base:base + n] = np.arange(a, b)
            if n < k * POOL_SUB:
                # duplicate member nodes (cycled, unbiased sample) into pads
                npad = k * POOL_SUB - n
                node_of_col[base + n:base + k * POOL_SUB] = \
                    a + (np.arange(npad) % n)
                total_dup += npad
        cores.append(dict(lo=int(lo), hi=int(hi), col_of_node=col_of_node,
                          node_of_col=node_of_col, gorder=gorder))

    # ---- slot lists per core, grouped by dst window ----
    # per-window chunk counts, max'd across cores for SPMD
    win_cnt = np.zeros((N_CORES, NW), dtype=np.int64)
    per_core_slots = []
    for c in range(N_CORES):
        lo, hi = cores[c]["lo"], cores[c]["hi"]
        col_of_node = cores[c]["col_of_node"]
        node_of_col = cores[c]["node_of_col"]
        m = (dst_all >= lo) & (dst_all < hi)
        e_src = src_all[m]
        e_dstcol = col_of_node[dst_all[m] - lo]
        e_scale = dinv[e_src] * dinv[dst_all[m]]
        # self loops / duplicate columns: every real column col gets a copy of
        # the slot list of node_of_col[col]; for original columns that is just
        # its own edges (already present) + self loop. For duplicate columns we
        # must copy the source node's edges too.
        real = np.nonzero(node_of_col >= 0)[0]
        own_col = col_of_node[node_of_col[real]]
        is_dup = own_col != real
        # self loops for every real column
        s_src = node_of_col[real] + lo
        s_dstcol = real
        s_scale = dinv[s_src] ** 2
        # duplicated edges: for dup column d with source node v, copy v's edges
        dup_cols = real[is_dup]
        dup_nodes = node_of_col[dup_cols] + lo
        # gather each dup node's edge list
        order = np.argsort(dst_all[m], kind="stable")
        e_dst_sorted = dst_all[m][order]
        e_src_sorted = e_src[order]
        estart = np.searchsorted(e_dst_sorted, np.arange(lo, hi + 1))
        d_src_list = []
        d_dstcol_list = []
        d_scale_list = []
        for dcol, v in zip(dup_cols, dup_nodes):
            a, b = estart[v - lo], estart[v - lo + 1]
            d_src_list.append(e_src_sorted[a:b])
            d_dstcol_list.append(np.full(b - a, dcol, dtype=np.int64))
            d_scale_list.append(dinv[e_src_sorted[a:b]] * dinv[v])
        d_src = np.concatenate(d_src_list) if d_src_list else np.zeros(0, np.int64)
        d_dstcol = (np.concatenate(d_dstcol_list) if d_dstcol_list
                    else np.zeros(0, np.int64))
        d_scale = (np.concatenate(d_scale_list) if d_scale_list
                   else np.zeros(0, np.float64))

        slot_src = np.concatenate([e_src, s_src, d_src])
        slot_dstcol = np.concatenate([e_dstcol, s_dstcol, d_dstcol])
        slot_scale = np.concatenate([e_scale, s_scale, d_scale])
        w = slot_dstcol // WIN
        win_cnt[c] += np.bincount(w, minlength=NW)
        per_core_slots.append((slot_src, slot_dstcol, slot_scale, w))

    win_chunks = np.maximum(np.ceil(win_cnt / CHUNK).astype(np.int64).max(axis=0), 1)
    # chunks per tile (TILE_DST/WIN windows each)
    WPT = TILE_DST // WIN
    tile_chunks = win_chunks.reshape(NT, WPT).sum(axis=1)
    TC = int(win_chunks.sum())
    chunk_off = np.zeros(NW + 1, dtype=np.int64)
    chunk_off[1:] = np.cumsum(win_chunks)

    # s-vector per column (sum of slot scales), per core
    for c in range(N_CORES):
        slot_src, slot_dstcol, slot_scale, w = per_core_slots[c]
        s_vec = np.zeros(SH, dtype=np.float64)
        np.add.at(s_vec, slot_dstcol, slot_scale)
        cores[c]["s_vec"] = s_vec.astype(np.float32)

        # scatter slots into padded chunk layout
        idx1 = np.zeros((TC, CHUNK), dtype=np.int64)   # src input node id
        dstr = np.full((TC, CHUNK), -1.0, dtype=np.float32)  # dst rel to window
        scl = np.zeros((TC, CHUNK), dtype=np.float32)
        order = np.argsort(w, kind="stable")
        ws = w[order]
        srcs = slot_src[order]
        dcols = slot_dstcol[order]
        scls = slot_scale[order]
        wstart = np.searchsorted(ws, np.arange(NW + 1))
        for wi in range(NW):
            a, b = wstart[wi], wstart[wi + 1]
            n = b - a
            base = chunk_off[wi] * CHUNK
            fl_idx = base + np.arange(n)
            idx1.reshape(-1)[fl_idx] = srcs[a:b]
            dstr.reshape(-1)[fl_idx] = (dcols[a:b] - wi * WIN).astype(np.float32)
            scl.reshape(-1)[fl_idx] = scls[a:b].astype(np.float32)
        cores[c]["idx_l1"] = np.ascontiguousarray(idx1.T).astype(np.int32)  # [128, TC]
        # l2 rows (owner*SH + owner-local column) filled in a second pass
        cores[c]["_l2_pending"] = (owner_of_node[idx1.reshape(-1)], idx1)

        cores[c]["dstr"] = np.ascontiguousarray(dstr.T)  # [128, TC] f32
        cores[c]["scl"] = np.ascontiguousarray(scl.T)

    # l2 rows need every core's col_of_node
    for c in range(N_CORES):
        own, idx1 = cores[c].pop("_l2_pending")
        flat = idx1.reshape(-1)
        l2rows = np.zeros(TC * CHUNK, dtype=np.int64)
        for oc in range(N_CORES):
            mm = own == oc
            if not mm.any():
                continue
            loc = flat[mm] - node_start[oc]
            l2rows[mm] = oc * SH + cores[oc]["col_of_node"][loc]
        cores[c]["idx_l2"] = np.ascontiguousarray(
            l2rows.reshape(TC, CHUNK).T).astype(np.int32)

    n_eff = N_NODES + total_dup          # real + duplicate columns
    n_pad_eff = N_CORES * SH - n_eff     # dummy/tail columns (relu(bias) valued)

    return dict(
        cores=cores, NT=NT, SH=SH, NW=NW, TC=TC,
        win_chunks=win_chunks, tile_chunks=tile_chunks, chunk_off=chunk_off,
        caps=caps, KMAX=KMAX, G_PAD=G_PAD, bucket_col=bucket_col,
        bucket_g=bucket_g, node_start=node_start,
        n_eff=int(n_eff), n_pad_eff=int(n_pad_eff), total_dup=int(total_dup),
    )


def make_cfg_inputs(plan, inputs):
    """Build kernel cfg + per-core in_maps + host assemble()."""
    NT, SH, TC = plan["NT"], plan["SH"], plan["TC"]
    G_PAD = plan["G_PAD"]
    G_OUT = int(np.ceil(G_PAD / 128) * 128)
    KC_CELL = [128] * (DIM_CELL_PAD // 128)

    cfg = dict(
        NT=NT, SH=SH, TC=TC, G_PAD=G_PAD, G_OUT=G_OUT,
        win_chunks=tuple(int(x) for x in plan["win_chunks"]),
        tile_chunks=tuple(int(x) for x in plan["tile_chunks"]),
        caps=tuple(int(x) for x in plan["caps"]),
        KMAX=plan["KMAX"], KC_CELL=KC_CELL,
        n_eff=plan["n_eff"], n_pad_eff=plan["n_pad_eff"],
    )

    import ml_dtypes
    x_bf16 = np.ascontiguousarray(
        np.asarray(inputs["drug_feature"], dtype=np.float32)).astype(ml_dtypes.bfloat16)
    gex = np.zeros((N_GRAPHS, DIM_CELL_PAD), dtype=np.float32)
    gex[:, :DIM_CELL] = np.asarray(inputs["gexpr_data"], dtype=np.float32)
    gex_bf16 = gex.astype(ml_dtypes.bfloat16)
    Wc1_pad = np.zeros((DIM_CELL_PAD, HID), dtype=np.float32)
    Wc1_pad[:DIM_CELL] = np.asarray(inputs["Wc1"], dtype=np.float32)

    def bf16(a):
        return np.ascontiguousarray(np.asarray(a, dtype=np.float32)).astype(ml_dtypes.bfloat16)

    def f32(a):
        return np.ascontiguousarray(np.asarray(a, dtype=np.float32))

    w_bf16 = dict(W1=bf16(inputs["W1"]), W2=bf16(inputs["W2"]),
                  Wc1=bf16(Wc1_pad), Wc2=bf16(inputs["Wc2"]))
    w_f32 = {k: f32(inputs[k]) for k in
             ("b1", "g1", "be1", "b2", "g2", "be2", "bc1", "gc1", "bec1", "bc2")}

    in_maps = []
    for c in range(N_CORES):
        core = plan["cores"][c]
        m = dict(
            x_full=x_bf16,
            idx_l1=core["idx_l1"], idx_l2=core["idx_l2"],
            dstr=core["dstr"].astype(ml_dtypes.bfloat16),
            scl=core["scl"].astype(ml_dtypes.bfloat16),
            s_rows=core["s_vec"].reshape(1, -1).astype(ml_dtypes.bfloat16),
            gexpr=np.ascontiguousarray(gex_bf16[c * GPC:(c + 1) * GPC]),
            **w_bf16, **w_f32,
        )
        in_maps.append(m)

    gorders = [plan["cores"][c]["gorder"] for c in range(N_CORES)]

    def assemble(results):
        x_drug = np.full((N_GRAPHS, OUT), -np.inf, dtype=np.float32)
        x_cell = np.empty((N_GRAPHS, OUT), dtype=np.float32)
        for c in range(N_CORES):
            o = np.asarray(results[c]["out"])
            gorder = gorders[c]
            valid = np.nonzero(gorder >= 0)[0]
            x_drug[c * GPC + gorder[valid]] = o[valid]
            x_cell[c * GPC:(c + 1) * GPC] = o[G_OUT:G_OUT + GPC]
        return x_drug, x_cell

    return cfg, in_maps, assemble


# ---------------------------------------------------------------------------
# kernel
# ---------------------------------------------------------------------------

class _PartDone(Exception):
    pass


F32 = mybir.dt.float32
BF16 = mybir.dt.bfloat16
I32 = mybir.dt.int32
AF = mybir.ActivationFunctionType
ALU = mybir.AluOpType


def build_kernel(cfg):
    NT, SH, TC = cfg["NT"], cfg["SH"], cfg["TC"]
    G_PAD, G_OUT = cfg["G_PAD"], cfg["G_OUT"]
    win_chunks = cfg["win_chunks"]
    tile_chunks = cfg["tile_chunks"]
    caps, KMAX = cfg["caps"], cfg["KMAX"]
    KCs = cfg["KC_CELL"]
    N_EFF, NPAD_EFF = cfg["n_eff"], cfg["n_pad_eff"]
    TD = TILE_DST
    WPT = TD // WIN
    HH = OUT // 128
    NWRITE = 4                  # dst tiles per u1 write DMA
    parts = cfg.get("parts", "all")

    nc = bacc.Bacc(None, num_devices=N_CORES)

    # parameters
    x_full = nc.declare_dram_parameter("x_full", [N_NODES, DIM_DRUG], BF16, isOutput=False)
    idx_l1 = nc.declare_dram_parameter("idx_l1", [CHUNK, TC], I32, isOutput=False)
    idx_l2 = nc.declare_dram_parameter("idx_l2", [CHUNK, TC], I32, isOutput=False)
    dstr_p = nc.declare_dram_parameter("dstr", [CHUNK, TC], BF16, isOutput=False)
    scl_p = nc.declare_dram_parameter("scl", [CHUNK, TC], BF16, isOutput=False)
    s_rows = nc.declare_dram_parameter("s_rows", [1, SH], BF16, isOutput=False)
    gexpr = nc.declare_dram_parameter("gexpr", [GPC, DIM_CELL_PAD], BF16, isOutput=False)
    W1p = nc.declare_dram_parameter("W1", [DIM_DRUG, HID], BF16, isOutput=False)
    W2p = nc.declare_dram_parameter("W2", [HID, OUT], BF16, isOutput=False)
    Wc1p = nc.declare_dram_parameter("Wc1", [DIM_CELL_PAD, HID], BF16, isOutput=False)
    Wc2p = nc.declare_dram_parameter("Wc2", [HID, OUT], BF16, isOutput=False)
    b1p = nc.declare_dram_parameter("b1", [HID], F32, isOutput=False)
    g1p = nc.declare_dram_parameter("g1", [HID], F32, isOutput=False)
    be1p = nc.declare_dram_parameter("be1", [HID], F32, isOutput=False)
    b2p = nc.declare_dram_parameter("b2", [OUT], F32, isOutput=False)
    g2p = nc.declare_dram_parameter("g2", [OUT], F32, isOutput=False)
    be2p = nc.declare_dram_parameter("be2", [OUT], F32, isOutput=False)
    bc1p = nc.declare_dram_parameter("bc1", [HID], F32, isOutput=False)
    gc1p = nc.declare_dram_parameter("gc1", [HID], F32, isOutput=False)
    bec1p = nc.declare_dram_parameter("bec1", [HID], F32, isOutput=False)
    bc2p = nc.declare_dram_parameter("bc2", [OUT], F32, isOutput=False)
    out = nc.declare_dram_parameter("out", [G_OUT + GPC, OUT], F32, isOutput=True)

    # internal DRAM
    u1_shard = nc.dram_tensor("u1_shard", [SH, HID], BF16)
    u1_full = nc.dram_tensor("u1_full", [N_CORES * SH, HID], BF16, addr_space="Shared")
    st1_in = nc.dram_tensor("st1_in", [128, 4], F32)
    st1_out = nc.dram_tensor("st1_out", [128, 4], F32, addr_space="Shared")
    st2_in = nc.dram_tensor("st2_in", [128, 4], F32)
    st2_out = nc.dram_tensor("st2_out", [128, 4], F32, addr_space="Shared")

    rg = [list(range(N_CORES))]

    with tile.TileContext(nc) as tc, ExitStack() as ctx:
      try:
        cpool = ctx.enter_context(tc.tile_pool(name="consts", bufs=1))
        meta_p = ctx.enter_context(tc.tile_pool(name="meta", bufs=1))
        rows_p = ctx.enter_context(tc.tile_pool(name="rows", bufs=2))
        p_p = ctx.enter_context(tc.tile_pool(name="onehot", bufs=2))
        work_p = ctx.enter_context(tc.tile_pool(name="work", bufs=3))
        stage_p = ctx.enter_context(tc.tile_pool(name="stage", bufs=2))
        stats_p = ctx.enter_context(tc.tile_pool(name="stats", bufs=1))
        cell_p = ctx.enter_context(tc.tile_pool(name="cell", bufs=2))
        u2_p = ctx.enter_context(tc.tile_pool(name="u2", bufs=1))
        psMM = ctx.enter_context(tc.tile_pool(name="psMM", bufs=3, space="PSUM"))
        psU = ctx.enter_context(tc.tile_pool(name="psU", bufs=2, space="PSUM"))
        psTR = ctx.enter_context(tc.tile_pool(name="psTR", bufs=1, space="PSUM"))

        # ---------------- constants ----------------
        ident = cpool.tile([128, 128], F32)
        make_identity(nc, ident[:])
        identb = cpool.tile([128, 128], BF16)
        nc.vector.tensor_copy(identb[:], ident[:])
        iota_i = cpool.tile([128, WIN], I32)
        nc.gpsimd.iota(iota_i[:], pattern=[[1, WIN]], base=0, channel_multiplier=0)
        iota_b = cpool.tile([128, WIN], BF16)
        nc.vector.tensor_copy(iota_b[:], iota_i[:])

        W1 = cpool.tile([128, HID], BF16)
        nc.sync.dma_start(out=W1[:], in_=W1p[:, :])
        W2 = cpool.tile([128, OUT], BF16)
        nc.sync.dma_start(out=W2[:], in_=W2p[:, :])
        W2f = cpool.tile([128, OUT], BF16)
        Wc2 = cpool.tile([128, OUT], BF16)
        nc.sync.dma_start(out=Wc2[:], in_=Wc2p[:, :])
        Wc2f = cpool.tile([128, OUT], BF16)
        Wc1t = []
        koff = 0
        for ki, kk in enumerate(KCs):
            w = cpool.tile([128, HID], BF16, tag=f"wc1_{ki}")
            nc.sync.dma_start(out=w[:kk, :], in_=Wc1p[koff:koff + kk, :])
            Wc1t.append(w)
            koff += kk

        def col(param, n=128, off=0):
            t = cpool.tile([n, 1], F32, tag=f"col_{param.name}_{off}")
            nc.sync.dma_start(out=t[:], in_=param[off:off + n, None])
            return t

        b1c, g1c, be1c = col(b1p), col(g1p), col(be1p)
        b2c = [col(b2p, off=h * 128) for h in range(HH)]
        g2c = [col(g2p, off=h * 128) for h in range(HH)]
        be2c = [col(be2p, off=h * 128) for h in range(HH)]
        bc1c, gc1c, bec1c = col(bc1p), col(gc1p), col(bec1p)
        bc2c = [col(bc2p, off=h * 128) for h in range(HH)]

        # metadata preload (idx slot shared between layers)
        dstr_sb = meta_p.tile([128, TC], BF16)
        nc.sync.dma_start(out=dstr_sb[:], in_=dstr_p[:, :])
        scl_sb = meta_p.tile([128, TC], BF16)
        nc.sync.dma_start(out=scl_sb[:], in_=scl_p[:, :])

        # stats accumulators
        st1_sum = stats_p.tile([128, NT], F32)
        st1_sq = stats_p.tile([128, NT], F32)
        st2_sum = stats_p.tile([128, HH * NT], F32)
        st2_sq = stats_p.tile([128, HH * NT], F32)
        stc_sum = stats_p.tile([128, 2], F32)
        stc_sq = stats_p.tile([128, 2], F32)

        # layer-2 output, transposed + 8:1 pre-max-reduced, SBUF-resident.
        # graph columns are POOL_SUB-aligned so 8-groups never cross graphs.
        u2R = u2_p.tile([128, HH, SH // POOL_SUB], BF16)

        # chunk -> (tile-relative psum slice) bookkeeping
        # for tile t: windows w0..w0+WPT-1, chunks per window win_chunks[w]
        tile_chunk_off = [0]
        for t in range(NT):
            tile_chunk_off.append(tile_chunk_off[-1] + tile_chunks[t])
        gC_max = max(
            tile_chunk_off[min(tg + GATHER_TILES, NT)] - tile_chunk_off[tg]
            for tg in range(0, NT, GATHER_TILES))

        def agg_tiles(idx_param, post, tag):
            """Aggregation over all dst tiles; gathers GATHER_TILES tiles per
            indirect DMA; builds one-hot per tile in 2 DVE ops; one matmul
            per chunk into a [128, TD] psum."""
            idx_sb = meta_p.tile([128, TC], I32, tag="idx")
            nc.sync.dma_start(out=idx_sb[:], in_=idx_param[:, :])
            for tg in range(0, NT, GATHER_TILES):
                tiles = range(tg, min(tg + GATHER_TILES, NT))
                c0 = tile_chunk_off[tg]
                cN = tile_chunk_off[tiles[-1] + 1]
                gC = cN - c0
                rows = rows_p.tile([128, gC_max * CHUNK], BF16, tag=f"rows{tag}")
                table = x_full if idx_param is idx_l1 else u1_full
                for cc in range(gC):
                    nc.gpsimd.indirect_dma_start(
                        out=rows[:, cc * CHUNK:(cc + 1) * CHUNK],
                        out_offset=None,
                        in_=table[:, :],
                        in_offset=bass.IndirectOffsetOnAxis(
                            ap=idx_sb[:, c0 + cc:c0 + cc + 1], axis=0),
                    )
                if parts == "gather" and tg == 0:
                    for q in range(8):
                        tf = work_p.tile([128, 128], F32, tag="dbgg")
                        nc.vector.tensor_copy(tf[:], rows[:, q * 128:(q + 1) * 128])
                        nc.sync.dma_start(out=out[q * 128:(q + 1) * 128, :128],
                                          in_=tf[:])
                    raise _PartDone()
                for t in tiles:
                    t0 = tile_chunk_off[t]
                    Ct = tile_chunks[t]
                    # one-hot, scaled: P[p, j, d] = (dstr[p, t0+j] == d) * scl
                    P = p_p.tile([128, Ct, WIN], BF16, tag=f"P{tag}")
                    nc.vector.tensor_tensor(
                        out=P[:],
                        in0=iota_b[:, None, :].to_broadcast([128, Ct, WIN]),
                        in1=dstr_sb[:, t0:t0 + Ct, None].to_broadcast([128, Ct, WIN]),
                        op=ALU.is_equal)
                    nc.vector.tensor_tensor(
                        out=P[:], in0=P[:],
                        in1=scl_sb[:, t0:t0 + Ct, None].to_broadcast([128, Ct, WIN]),
                        op=ALU.mult)
                    zT = psMM.tile([128, TD], F32, tag="mm", space="PSUM")
                    j = 0
                    for wi in range(WPT):
                        nw = win_chunks[t * WPT + wi]
                        for k in range(nw):
                            nc.tensor.matmul(
                                zT[:, wi * WIN:(wi + 1) * WIN],
                                lhsT=rows[:, (t0 - c0 + j) * CHUNK:
                                          (t0 - c0 + j + 1) * CHUNK],
                                rhs=P[:, j, :],
                                start=(k == 0), stop=(k == nw - 1))
                            j += 1
                    post(t, zT)

        # ---------------- layer 1 ----------------
        u1_stage = [None]

        def l1_post(t, zT):
            zs = work_p.tile([128, TD], BF16, tag="zs1")
            nc.vector.tensor_copy(zs[:], zT[:])
            uTf = psU.tile([128, 2 * TD], F32, tag="u2", space="PSUM")
            uT = uTf[:, :TD]
            nc.tensor.matmul(uT, lhsT=W1[:], rhs=zs[:], start=True, stop=True)
            u1t = work_p.tile([128, TD], BF16, tag="u1t")
            nc.scalar.activation(u1t[:], uT, AF.Relu, bias=b1c[:],
                                 accum_out=st1_sum[:, t:t + 1])
            sq = work_p.tile([128, TD], BF16, tag="sq1")
            nc.scalar.activation(sq[:], u1t[:], AF.Square,
                                 accum_out=st1_sq[:, t:t + 1])
            if t % NWRITE == 0:
                u1_stage[0] = stage_p.tile([128, NWRITE * 2, 128], BF16,
                                           tag="u1st", name="u1st")
            stg = u1_stage[0]
            for h in range(TD // 128):
                pt = psTR.tile([128, 128], BF16, tag="trb", space="PSUM", bufs=2)
                nc.tensor.transpose(pt[:], u1t[:, h * 128:(h + 1) * 128], identb[:])
                nc.vector.tensor_copy(stg[:, (t % NWRITE) * 2 + h, :], pt[:])
            if t % NWRITE == NWRITE - 1 or t == NT - 1:
                nt = t % NWRITE + 1
                r0 = (t - nt + 1) * TD
                nc.sync.dma_start(
                    out=u1_shard[r0:r0 + nt * TD, :]
                        .rearrange("(q p) f -> p q f", p=128),
                    in_=stg[:, :nt * 2, :])

        agg_tiles(idx_l1, l1_post, "a")

        if parts == "l1":
            for q in range(8):
                tt = work_p.tile([128, 128], BF16, tag="dbg")
                nc.sync.dma_start(out=tt[:], in_=u1_shard[q * 128:(q + 1) * 128, :])
                ttf = work_p.tile([128, 128], F32, tag="dbgf")
                nc.vector.tensor_copy(ttf[:], tt[:])
                nc.sync.dma_start(out=out[q * 128:(q + 1) * 128, :128], in_=ttf[:])
            raise _PartDone()

        st1_red = stats_p.tile([128, 4], F32)
        nc.vector.tensor_reduce(st1_red[:, 0:1], st1_sum[:, :], mybir.AxisListType.X, ALU.add)
        nc.vector.tensor_reduce(st1_red[:, 1:2], st1_sq[:, :], mybir.AxisListType.X, ALU.add)

        # ---------------- cell pass 1 (tanh + stats) ----------------
        n_bh = GPC // 512
        cT = []
        for bh in range(n_bh):
            pc = psU.tile([128, 512], F32, tag="u2", space="PSUM")
            koff = 0
            for ki, kk in enumerate(KCs):
                rhs = cell_p.tile([128, 512], BF16, tag=f"cellrhs")
                nc.sync.dma_start(
                    out=rhs[:kk, :],
                    in_=gexpr[bh * 512:(bh + 1) * 512, koff:koff + kk],
                    transpose=True)
                nc.tensor.matmul(pc[:], lhsT=Wc1t[ki][:kk, :], rhs=rhs[:kk, :],
                                 start=(ki == 0), stop=(ki == len(KCs) - 1))
                koff += kk
            ct = cell_p.tile([128, 512], BF16, tag="cellct")
            nc.scalar.activation(ct[:], pc[:], AF.Tanh, bias=bc1c[:],
                                 accum_out=stc_sum[:, bh:bh + 1])
            csq = cell_p.tile([128, 512], BF16, tag="cellsq")
            nc.scalar.activation(csq[:], ct[:], AF.Square,
                                 accum_out=stc_sq[:, bh:bh + 1])
            cT.append(ct)
        nc.vector.tensor_reduce(st1_red[:, 2:3], stc_sum[:, :], mybir.AxisListType.X, ALU.add)
        nc.vector.tensor_reduce(st1_red[:, 3:4], stc_sq[:, :], mybir.AxisListType.X, ALU.add)
        nc.sync.dma_start(out=st1_in[:, :], in_=st1_red[:])

        # AR1 + AG
        tc.strict_bb_all_engine_barrier()
        nc.gpsimd.collective_compute(
            "AllReduce", ALU.add, replica_groups=rg,
            ins=[st1_in[:]], outs=[st1_out[:]])
        nc.gpsimd.collective_compute(
            "AllGather", ALU.bypass, replica_groups=rg,
            ins=[u1_shard[:]], outs=[u1_full[:]])
        st1_sb = stats_p.tile([128, 4], F32)
        nc.sync.dma_start(out=st1_sb[:], in_=st1_out[:, :])

        # ---------------- BN affines ----------------
        def bn_affine(sum_c, sq_c, gc, bec, bias_relu_col, n_real, n_pad, pfx):
            a_c = cpool.tile([128, 1], F32, tag=f"{pfx}_a")
            c_c = cpool.tile([128, 1], F32, tag=f"{pfx}_c")
            m_c = cpool.tile([128, 1], F32, tag=f"{pfx}_m")
            q_c = cpool.tile([128, 1], F32, tag=f"{pfx}_q")
            t1 = cpool.tile([128, 1], F32, tag=f"{pfx}_t1")
            if bias_relu_col is not None:
                rb = cpool.tile([128, 1], F32, tag=f"{pfx}_rb")
                nc.scalar.activation(rb[:], bias_relu_col[:], AF.Relu)
                rb2 = cpool.tile([128, 1], F32, tag=f"{pfx}_rb2")
                nc.scalar.activation(rb2[:], rb[:], AF.Square)
                nc.vector.tensor_scalar(out=m_c[:], in0=rb[:], scalar1=float(-n_pad),
                                        scalar2=None, op0=ALU.mult)
                nc.vector.tensor_tensor(out=m_c[:], in0=m_c[:], in1=sum_c, op=ALU.add)
                nc.vector.tensor_scalar(out=m_c[:], in0=m_c[:], scalar1=1.0 / n_real,
                                        scalar2=None, op0=ALU.mult)
                nc.vector.tensor_scalar(out=q_c[:], in0=rb2[:], scalar1=float(-n_pad),
                                        scalar2=None, op0=ALU.mult)
                nc.vector.tensor_tensor(out=q_c[:], in0=q_c[:], in1=sq_c, op=ALU.add)
                nc.vector.tensor_scalar(out=q_c[:], in0=q_c[:], scalar1=1.0 / n_real,
                                        scalar2=None, op0=ALU.mult)
            else:
                nc.vector.tensor_scalar(out=m_c[:], in0=sum_c, scalar1=1.0 / n_real,
                                        scalar2=None, op0=ALU.mult)
                nc.vector.tensor_scalar(out=q_c[:], in0=sq_c, scalar1=1.0 / n_real,
                                        scalar2=None, op0=ALU.mult)
            nc.scalar.activation(t1[:], m_c[:], AF.Square)
            nc.vector.tensor_tensor(out=t1[:], in0=q_c[:], in1=t1[:], op=ALU.subtract)
            nc.vector.tensor_scalar(out=t1[:], in0=t1[:], scalar1=float(EPS),
                                    scalar2=None, op0=ALU.add)
            nc.vector.reciprocal(t1[:], t1[:])
            nc.scalar.activation(t1[:], t1[:], AF.Sqrt)
            nc.vector.tensor_tensor(out=a_c[:], in0=gc[:], in1=t1[:], op=ALU.mult)
            nc.vector.tensor_tensor(out=c_c[:], in0=m_c[:], in1=a_c[:], op=ALU.mult)
            nc.vector.tensor_tensor(out=c_c[:], in0=bec[:], in1=c_c[:], op=ALU.subtract)
            return a_c, c_c

        a1c, c1c = bn_affine(st1_sb[:, 0:1], st1_sb[:, 1:2], g1c, be1c, b1c,
                             N_EFF, NPAD_EFF, "bn1")
        acc_, ccc_ = bn_affine(st1_sb[:, 2:3], st1_sb[:, 3:4], gc1c, bec1c, None,
                               N_GRAPHS, 0, "bnc")

        # W2' = a1*W2 ; r2 rows via PE outer products
        nc.vector.tensor_scalar(out=W2f[:], in0=W2[:], scalar1=a1c[:],
                                scalar2=None, op0=ALU.mult)
        c1b = cpool.tile([128, 1], BF16, tag="c1b")
        nc.vector.tensor_copy(c1b[:], c1c[:])
        r2 = []
        for h in range(HH):
            pr = psTR.tile([128, 128], F32, tag="tr", space="PSUM")
            nc.tensor.matmul(pr[:1, :], lhsT=c1b[:], rhs=W2[:, h * 128:(h + 1) * 128],
                             start=True, stop=True)
            rr = cpool.tile([1, 128], BF16, tag=f"r2_{h}")
            nc.vector.tensor_copy(rr[:], pr[:1, :])
            r2.append(rr)

        # cell finish
        nc.vector.tensor_scalar(out=Wc2f[:], in0=Wc2[:], scalar1=acc_[:],
                                scalar2=None, op0=ALU.mult)
        ccb = cpool.tile([128, 1], BF16, tag="ccb")
        nc.vector.tensor_copy(ccb[:], ccc_[:])
        bc2f = []
        for h in range(HH):
            pb = psTR.tile([128, 128], F32, tag="tr", space="PSUM")
            nc.tensor.matmul(pb[:, :1], lhsT=Wc2[:, h * 128:(h + 1) * 128], rhs=ccb[:],
                             start=True, stop=True)
            bb_ = cpool.tile([128, 1], F32, tag=f"bc2f_{h}")
            nc.vector.tensor_tensor(out=bb_[:], in0=pb[:, :1], in1=bc2c[h][:], op=ALU.add)
            bc2f.append(bb_)
        for bh in range(n_bh):
            for h in range(HH):
                px = psU.tile([128, 512], F32, tag="u2", space="PSUM")
                nc.tensor.matmul(px[:], lhsT=Wc2f[:, h * 128:(h + 1) * 128],
                                 rhs=cT[bh][:], start=True, stop=True)
                xc = cell_p.tile([128, 512], BF16, tag="cellxc")
                nc.scalar.activation(xc[:], px[:], AF.Relu, bias=bc2f[h][:])
                stg = stage_p.tile([128, 4, 128], F32, tag="cellst")
                for bb in range(4):
                    pt = psTR.tile([128, 128], BF16, tag="trb", space="PSUM", bufs=2)
                    nc.tensor.transpose(pt[:], xc[:, bb * 128:(bb + 1) * 128], identb[:])
                    nc.vector.tensor_copy(stg[:, bb, :], pt[:])
                nc.sync.dma_start(
                    out=out[G_OUT + bh * 512:G_OUT + (bh + 1) * 512,
                            h * 128:(h + 1) * 128]
                        .rearrange("(q p) f -> p q f", p=128),
                    in_=stg[:])

        if parts == "l1c":
            raise _PartDone()

        # ---------------- layer 2 ----------------
        s_sup = [None]

        def l2_post(t, zT):
            zs = work_p.tile([128, TD], BF16, tag="zs2")
            nc.vector.tensor_copy(zs[:], zT[:])
            if t % 8 == 0:
                s_sup[0] = stage_p.tile([1, 8 * TD], BF16, tag="s_sup", name="s_sup")
                n_here = min(8 * TD, SH - t * TD)
                nc.sync.dma_start(out=s_sup[0][:, :n_here],
                                  in_=s_rows[:, t * TD:t * TD + n_here])
            st_ = s_sup[0]
            uT = psU.tile([128, 2 * TD], F32, tag="u2", space="PSUM")
            u2t = work_p.tile([128, HH, TD], BF16, tag="u2t")
            for h in range(HH):
                nc.tensor.matmul(uT[:, h * TD:(h + 1) * TD],
                                 lhsT=W2f[:, h * 128:(h + 1) * 128],
                                 rhs=zs[:], start=True, stop=False)
                nc.tensor.matmul(uT[:, h * TD:(h + 1) * TD],
                                 lhsT=r2[h][:],
                                 rhs=st_[:, (t % 8) * TD:(t % 8 + 1) * TD],
                                 start=False, stop=True)
                nc.scalar.activation(u2t[:, h, :],
                                     uT[:, h * TD:(h + 1) * TD], AF.Relu,
                                     bias=b2c[h][:],
                                     accum_out=st2_sum[:, t * HH + h:t * HH + h + 1])
                sq = work_p.tile([128, TD], BF16, tag="sq2")
                nc.scalar.activation(sq[:], u2t[:, h, :], AF.Square,
                                     accum_out=st2_sq[:, t * HH + h:t * HH + h + 1])
            TD8 = TD // POOL_SUB
            nc.vector.tensor_reduce(
                u2R[:, :, t * TD8:(t + 1) * TD8],
                u2t[:].rearrange("p h (g l) -> p h g l", l=POOL_SUB),
                mybir.AxisListType.X, ALU.max)

        agg_tiles(idx_l2, l2_post, "b")

        # ---------------- L2 stats + AR2 ----------------
        st2_red = stats_p.tile([128, 4], F32)
        for h in range(HH):
            nc.vector.tensor_reduce(
                st2_red[:, 2 * h:2 * h + 1],
                st2_sum[:].rearrange("p (t h) -> p t h", h=HH)[:, :, h],
                mybir.AxisListType.X, ALU.add)
            nc.vector.tensor_reduce(
                st2_red[:, 2 * h + 1:2 * h + 2],
                st2_sq[:].rearrange("p (t h) -> p t h", h=HH)[:, :, h],
                mybir.AxisListType.X, ALU.add)
        nc.sync.dma_start(out=st2_in[:, :], in_=st2_red[:])
        tc.strict_bb_all_engine_barrier()
        nc.gpsimd.collective_compute(
            "AllReduce", ALU.add, replica_groups=rg,
            ins=[st2_in[:]], outs=[st2_out[:]])
        st2_sb = stats_p.tile([128, 4], F32)
        nc.sync.dma_start(out=st2_sb[:], in_=st2_out[:, :])

        a2c, c2c = [], []
        for h in range(HH):
            a_, c_ = bn_affine(st2_sb[:, 2 * h:2 * h + 1], st2_sb[:, 2 * h + 1:2 * h + 2],
                               g2c[h], be2c[h], b2c[h], N_EFF, NPAD_EFF, f"bn2_{h}")
            a2c.append(a_)
            c2c.append(c_)

        if parts == "nopool":
            raise _PartDone()

        # ---------------- pool: uniform bucket reduces over u2R ----------------
        pooled = stats_p.tile([128, HH, G_OUT], BF16)
        col8_off = 0
        g_off = 0
        for k in range(1, KMAX + 1):
            capk = caps[k]
            if capk == 0:
                continue
            nc.vector.tensor_reduce(
                pooled[:, :, g_off:g_off + capk],
                u2R[:, :, col8_off:col8_off + capk * k]
                    .rearrange("p h (g l) -> p h g l", l=k),
                mybir.AxisListType.X, ALU.max)
            col8_off += capk * k
            g_off += capk
        for h in range(HH):
            for q in range(G_OUT // 128):
                n_here = min(128, G_PAD - q * 128)
                if n_here <= 0:
                    break
                blk = work_p.tile([128, 128], F32, tag="poolblk")
                nc.vector.tensor_scalar(
                    out=blk[:, :n_here], in0=pooled[:, h, q * 128:q * 128 + n_here],
                    scalar1=a2c[h][:], scalar2=c2c[h][:], op0=ALU.mult, op1=ALU.add)
                pt = psTR.tile([128, 128], F32, tag="tr", space="PSUM")
                nc.tensor.transpose(pt[:n_here, :], blk[:, :n_here], ident[:])
                rt = work_p.tile([128, 128], F32, tag="poolrt")
                nc.vector.tensor_copy(rt[:n_here, :], pt[:n_here, :])
                nc.sync.dma_start(
                    out=out[q * 128:q * 128 + n_here, h * 128:(h + 1) * 128],
                    in_=rt[:n_here, :])

      except _PartDone:
        pass
    nc.compile()
    return nc


_BUILD_CACHE = {}


def _run(inputs, trace=False, parts="all"):
    plan = build_plan(inputs["drug_adj"], inputs["ibatch"])
    cfg, in_maps, assemble = make_cfg_inputs(plan, inputs)
    cfg["parts"] = parts
    key = (cfg["NT"], cfg["TC"], cfg["G_PAD"], cfg["win_chunks"], cfg["caps"], parts)
    if key not in _BUILD_CACHE:
        _BUILD_CACHE[key] = build_kernel(cfg)
    nc = _BUILD_CACHE[key]
    res = run_bass_kernel_spmd(nc, in_maps, core_ids=list(range(8)), trace=trace)
    x_drug, x_cell = assemble(res.results)
    return (x_drug, x_cell), res


def kernel(**inputs):
    inputs = {k: np.asarray(v) for k, v in inputs.items()}
    (x_drug, x_cell), _ = _run(inputs, trace=False)
    return x_drug, x_cell
